# revision 19
# baseline (speedup 1.0000x reference)
"""Trainium2 Bass kernel for nn_DirectionVarEntropy.

Computes, per 14x14 patch and channel:
  - pixel-value entropy (256-bin histogram of round(x*255))
  - direction variance psi of 3x3-DCT sliding-window directional stds
  - richness = mean_c(psi_m * entropy)  ->  output (B, Hp, Wp)

Sharding: pure data parallel over batch, 2 images per core on 8 cores.

Per-core layout: 2048 spatial patches x 3 channels = 6144 patch-channels,
mapped to [128 partitions x 48 free segments]; seg s = t*3 + c where
t = spatial_patch // 128, partition p = spatial_patch % 128.

Entropy (phase 1): per-pixel own-bin counts c_p = #{q: pi_q == pi_p} via
195 circular-shift equality passes.  The DVE computes one bf16 is_equal
plane per shift; the accumulation over shifts runs on the otherwise-idle
PE as identity-stationary matmuls accumulating into PSUM (f32), freeing
the DVE from the 194 add passes.  Segments go in 3 groups of 16 so the
per-group count tile (16*196 f32 = 12.25 KB) fits in PSUM.  Then
  E = log2(196) - mean_p ln(c_p)/ln 2
which equals the dense-histogram entropy up to the reference's 1e-10
epsilon terms (~1e-6 relative).  The entropy tiles live in a scoped pool
released before the DCT phase so its work pool can run 4 segments per
block.

DCT part (phase 2): explicit 9 coefficient planes via separable 3-tap
convolutions, group sums / variances / psi in fp32 on DVE; ACT does
clamps (Relu), squares, and direct Sqrt.  Relu/Square/Sqrt coexist in
one activation table set, so only ~2 table loads total (the previous
exp(0.5*ln x) form thrashed between the natural_log and exp sets every
block, ~170 reloads at 1.3 us each).
"""

import functools

import numpy as np

import concourse.bacc as bacc
import concourse.bass as bass
import concourse.mybir as mybir
from concourse import bass_utils
from concourse.tile import TileContext

P = 128
PH = 14
NWIN = 12          # sliding 3x3 positions per axis
NPIX = PH * PH     # 196
BINS = 256
LN2 = 0.6931471805599453
F32 = mybir.dt.float32
BF16 = mybir.dt.bfloat16
ALU = mybir.AluOpType
ACTF = mybir.ActivationFunctionType

# problem shape (hardcoded per contract)
B_FULL, C, H, W = 16, 3, 448, 448
N_CORES = 8
B_CORE = B_FULL // N_CORES      # 2
HP = H // PH                    # 32
T_BLKS = B_CORE * HP * HP // P  # 16 t-blocks of 128 spatial patches
SEGS = T_BLKS * C               # 48
GSEG = 16                       # segs per entropy PSUM group
NGRP = SEGS // GSEG             # 3
GW = GSEG * NPIX                # 3136 f32 per group


def _build(dct_flat: tuple, segs: int = SEGS,
           nb: int = 3, mode: str = "full") -> bass.Bass:
    """Build the SPMD single-core program. dct_flat: 9 floats, row major."""
    D = np.asarray(dct_flat, np.float64).reshape(3, 3)
    nc = bacc.Bacc("TRN2", debug=False, enable_asserts=False)

    x_d = nc.dram_tensor("x", (B_CORE, C, H, W), F32, kind="ExternalInput")
    id_d = nc.dram_tensor("ident", (P, P), BF16, kind="ExternalInput")
    out_d = nc.dram_tensor("out", (B_CORE, HP, HP), F32, kind="ExternalOutput")
    xv = x_d.ap().rearrange("b c (hp i) (wp j) -> b c hp wp i j", i=PH, j=PH)
    ov = out_d.ap()

    n_blocks = (segs + nb - 1) // nb

    with TileContext(nc) as tc:
        with tc.tile_pool(name="persist", bufs=1) as pp:
            X = pp.tile([P, segs, PH, PH], F32)
            Xf = X.rearrange("p s i j -> p (s i j)")
            TMP = pp.tile([P, (segs // 8) * NPIX], F32)
            IDENT = pp.tile([P, P], BF16)
            dummy = pp.tile([P, NPIX], BF16)
            pdum = pp.tile([P, NWIN * NWIN], F32)
            psi_acc = pp.tile([P, segs], F32)
            e_acc = pp.tile([P, segs], F32)
            rich = pp.tile([P, segs], F32)
            rich3 = rich.rearrange("p (t c) -> p t c", c=C)
            tsum = pp.tile([P, segs // C], F32)
            osb = pp.tile([P, segs // C], F32)

            # ---- input DMAs: per (t, c, p1) a [32, 14, 14] strided load ----
            nc.sync.dma_start(IDENT, id_d.ap())
            for t in range(T_BLKS):
                b = t // (T_BLKS // B_CORE)
                hp0 = (t % (T_BLKS // B_CORE)) * 4
                for c in range(C):
                    s = t * C + c
                    for p1 in range(4):
                        nc.sync.dma_start(
                            X[p1 * 32:(p1 + 1) * 32, s],
                            xv[b, c, hp0 + p1],
                        )
            nc.vector.tensor_copy(IDENT, IDENT)
            d = [[float(D[r, c]) for c in range(3)] for r in range(3)]

            # ====== interleaved entropy (symmetric shifts) + DCT ======
            # Entropy: 98 is_equal planes per group instead of 195 — the PE
            # accumulates each plane twice (alignment 0 and a circular +s,
            # as two contiguous pieces) into a [GSEG, 256]-padded PSUM tile
            # (2 segs per bank; strided-out matmuls verified exact on HW).
            # The PE paces this phase, so the DCT block emission is a
            # generator pumped between eq planes: the DVE fills its
            # ring-wait stalls with DCT work instead of idling.
            ep_ctx = tc.tile_pool(name="ent", bufs=1)
            ep = ep_ctx.__enter__()
            NHALF = 98
            PI2 = ep.tile([P, segs, NPIX + NHALF], BF16)
            NEQ = 3
            EQR = [ep.tile([P, GSEG, NPIX], BF16, name=f"EQR{i}")
                   for i in range(NEQ)]
            LNP = ep.tile([P, GW], F32)
            LNP3 = LNP.rearrange("p (s k) -> p s k", k=NPIX)

            TWO23 = float(2 ** 23)
            qch = (segs // 8) * NPIX
            TMP3 = TMP.rearrange("p (s k) -> p s k", k=NPIX)
            spq = segs // 8
            loaded = [0]   # segs absorbed+quantized so far

            def load_chunks(q_lo, q_hi):
                for q in range(q_lo, q_hi):
                    for s in range(q * spq, (q + 1) * spq):
                        for p1 in range(4):
                            sl = X[p1 * 32:(p1 + 1) * 32, s]
                            nc.vector.tensor_copy(sl, sl)
                    nc.vector.tensor_scalar(
                        TMP, Xf[:, q * qch:(q + 1) * qch], 255.0, TWO23,
                        ALU.mult, ALU.add)
                    nc.vector.tensor_scalar(
                        PI2[:, q * spq:(q + 1) * spq, 0:NPIX], TMP3, TWO23,
                        None, ALU.subtract)
                    loaded[0] = (q + 1) * spq

            wp_ctx = tc.tile_pool(name="work", bufs=2)
            wp = wp_ctx.__enter__()
            gstate = {"need_seg": 0}

            def dct_blocks():
                GROUPS = (
                    [[(r, 0), (r, 1), (r, 2)] for r in range(3)]
                    + [[(0, c), (1, c), (2, c)] for c in range(3)]
                    + [[(0, 0), (1, 1), (2, 2)],
                       [(0, 2), (1, 1), (2, 0)]]
                )
                for blk in range(n_blocks):
                    s0 = blk * nb
                    sn = min(nb, segs - s0)
                    gstate["need_seg"] = s0 + sn
                    yield
                    V = [wp.tile([P, nb, NWIN, PH], F32, tag=f"V{r}",
                                 name=f"V{r}") for r in range(3)]
                    Y = [[wp.tile([P, nb, NWIN, NWIN], F32, tag=f"Y{r}{c}",
                                  name=f"Y{r}{c}") for c in range(3)]
                         for r in range(3)]
                    xb = X[:, s0:s0 + sn]
                    for r in range(3):
                        vb = V[r][:, :sn]
                        nc.vector.tensor_scalar(
                            vb, xb[:, :, 0:NWIN, :], d[r][0], None, ALU.mult)
                        for k in (1, 2):
                            nc.vector.scalar_tensor_tensor(
                                vb, xb[:, :, k:k + NWIN, :], d[r][k], vb,
                                ALU.mult, ALU.add)
                        yield
                    for r in range(3):
                        vb = V[r][:, :sn]
                        for c in range(3):
                            yb = Y[r][c][:, :sn]
                            nc.vector.tensor_scalar(
                                yb, vb[:, :, :, 0:NWIN], d[c][0], None,
                                ALU.mult)
                            for l in (1, 2):
                                nc.vector.scalar_tensor_tensor(
                                    yb, vb[:, :, :, l:l + NWIN], d[c][l], yb,
                                    ALU.mult, ALU.add)
                            yield
                    M = [wp.tile([P, nb, NWIN, NWIN], F32, tag=f"M{g}",
                                 name=f"M{g}") for g in range(8)]
                    SSP = [wp.tile([P, nb, NWIN, NWIN], F32, tag=f"SSP{i}",
                                   name=f"SSP{i}") for i in range(2)]
                    for g, mem in enumerate(GROUPS):
                        mb = M[g][:, :sn]
                        (r0, c0), (r1, c1), (r2, c2) = mem
                        nc.vector.tensor_add(
                            mb, Y[r0][c0][:, :sn], Y[r1][c1][:, :sn])
                        nc.vector.tensor_add(mb, mb, Y[r2][c2][:, :sn])
                        nc.scalar.activation(mb, mb, ACTF.Square,
                                             scale=1.0 / 3)
                        yield
                    for r in range(3):
                        for c in range(3):
                            yb = Y[r][c][:, :sn]
                            nc.scalar.activation(yb, yb, ACTF.Square)
                    yield
                    for g, mem in enumerate(GROUPS):
                        sb = SSP[g % 2][:, :sn]
                        mb = M[g][:, :sn]
                        (r0, c0), (r1, c1), (r2, c2) = mem
                        nc.vector.tensor_add(
                            sb, Y[r0][c0][:, :sn], Y[r1][c1][:, :sn])
                        nc.vector.tensor_add(sb, sb, Y[r2][c2][:, :sn])
                        nc.vector.scalar_tensor_tensor(
                            mb, sb, 1.0 / 3, mb, ALU.mult, ALU.subtract)
                        nc.scalar.activation(mb, mb, ACTF.Relu)
                        nc.scalar.activation(mb, mb, ACTF.Sqrt)
                        yield
                    U1, U2, t1 = Y[0][0], Y[0][1], Y[0][2]
                    t2, A, sum2 = Y[1][0], Y[1][1], Y[1][2]
                    aq, s_t, ssq = Y[2][0], Y[2][1], Y[2][2]
                    rinv, psi = SSP[0], SSP[1]
                    u1, u2 = U1[:, :sn], U2[:, :sn]
                    tb1, tb2 = t1[:, :sn], t2[:, :sn]
                    ab = A[:, :sn]
                    s2b, aqb = sum2[:, :sn], aq[:, :sn]
                    stb, ssqb, rb, psib = (s_t[:, :sn], ssq[:, :sn],
                                           rinv[:, :sn], psi[:, :sn])
                    sig = [M[g][:, :sn] for g in range(8)]
                    nc.vector.tensor_add(u1, sig[0], sig[1])
                    nc.vector.tensor_add(u1, u1, sig[2])
                    nc.vector.tensor_add(u2, sig[3], sig[4])
                    nc.vector.tensor_add(u2, u2, sig[5])
                    yield
                    nc.vector.scalar_tensor_tensor(
                        tb1, u1, 1.0 / 3, sig[6], ALU.mult, ALU.add)
                    nc.vector.scalar_tensor_tensor(
                        tb2, u2, 1.0 / 3, sig[7], ALU.mult, ALU.add)
                    nc.vector.tensor_add(ab, tb1, tb2)
                    yield
                    nc.scalar.activation(u1, u1, ACTF.Square, scale=1.0 / 3)
                    nc.scalar.activation(u2, u2, ACTF.Square, scale=1.0 / 3)
                    nc.scalar.activation(sig[6], sig[6], ACTF.Square)
                    nc.scalar.activation(sig[7], sig[7], ACTF.Square)
                    nc.vector.tensor_add(tb1, u1, u2)
                    nc.vector.tensor_add(tb2, sig[6], sig[7])
                    nc.vector.tensor_add(s2b, tb1, tb2)
                    yield
                    nc.scalar.activation(aqb, ab, ACTF.Square, scale=0.5)
                    nc.vector.tensor_sub(s2b, s2b, aqb)
                    nc.scalar.activation(stb, ab, ACTF.Copy, bias=1e-8,
                                         scale=0.25)
                    nc.scalar.activation(ssqb, stb, ACTF.Square)
                    nc.vector.reciprocal(rb, ssqb)
                    nc.vector.scalar_tensor_tensor(
                        psib, s2b, 1.0 / 3, rb, ALU.mult, ALU.mult)
                    yield
                    for i in range(sn):
                        s = s0 + i
                        nc.scalar.activation(
                            pdum, psib[:, i].rearrange("p i j -> p (i j)"),
                            ACTF.Copy, accum_out=psi_acc[:, s:s + 1])
                    yield

            dct_gen = dct_blocks() if mode != "ent_only" else None

            def pump(n):
                nonlocal_gen = None
                for _ in range(n):
                    if pump.gen is None:
                        return
                    if gstate["need_seg"] > loaded[0]:
                        return
                    try:
                        next(pump.gen)
                    except StopIteration:
                        pump.gen = None
                        return
            pump.gen = dct_gen

            n_half = NHALF if mode != "dct_only" else 2
            with tc.psum_pool(name="eps", bufs=1) as pq:
                ACCP = pq.tile([P, GSEG, 256], F32, name="ACCP")
                for g in range(NGRP):
                    q_hi = ((g + 1) * GSEG + spq - 1) // spq
                    q_lo = (g * GSEG + spq - 1) // spq
                    load_chunks(q_lo if g else 0, q_hi)
                    g0 = g * GSEG
                    nc.vector.tensor_copy(
                        PI2[:, g0:g0 + GSEG, NPIX:NPIX + NHALF],
                        PI2[:, g0:g0 + GSEG, 0:NHALF])
                    base_g = PI2[:, g0:g0 + GSEG, 0:NPIX]
                    for si, s in enumerate(range(1, 1 + n_half)):
                        eb = EQR[si % NEQ]
                        nc.vector.tensor_tensor(
                            eb, base_g, PI2[:, g0:g0 + GSEG, s:s + NPIX],
                            ALU.is_equal)
                        for b2 in range(GSEG // 2):
                            e2 = eb[:, 2 * b2:2 * b2 + 2]
                            a2 = ACCP[:, 2 * b2:2 * b2 + 2]
                            nc.tensor.matmul(
                                out=a2[:, :, 0:NPIX], lhsT=IDENT, rhs=e2,
                                start=(si == 0), stop=(si == n_half - 1))
                            if s < NHALF:
                                nc.tensor.matmul(
                                    out=a2[:, :, s:NPIX], lhsT=IDENT,
                                    rhs=e2[:, :, 0:NPIX - s],
                                    start=False, stop=False)
                                nc.tensor.matmul(
                                    out=a2[:, :, 0:s], lhsT=IDENT,
                                    rhs=e2[:, :, NPIX - s:NPIX],
                                    start=False, stop=False)
                        pump(2)
                    nc.scalar.activation(LNP, ACCP[:, :, 0:NPIX], ACTF.Ln,
                                         bias=1.0)
                    for i in range(GSEG):
                        s = g0 + i
                        nc.scalar.activation(
                            dummy, LNP3[:, i], ACTF.Copy,
                            accum_out=e_acc[:, s:s + 1])
                load_chunks(((NGRP - 1) * GSEG + GSEG + spq - 1) // spq, 8)
            # drain any remaining DCT work
            while pump.gen is not None:
                pump(64)
            wp_ctx.__exit__(None, None, None)
            ep_ctx.__exit__(None, None, None)
            if mode == "ent_only":
                nc.vector.memset(psi_acc, 1)

            # ---- richness = psi_m * entropy, mean over channels ----
            import math
            ln_n = float(math.log2(NPIX)) if mode != "dct_only" else 1.0
            sc = -1.0 / (NPIX * LN2) if mode != "dct_only" else 0.0
            nc.vector.tensor_scalar(
                e_acc, e_acc, sc, ln_n, ALU.mult, ALU.add)
            nc.vector.scalar_tensor_tensor(
                rich, psi_acc, 1.0 / (NWIN * NWIN), e_acc,
                ALU.mult, ALU.mult)
            nc.vector.tensor_add(tsum, rich3[:, :, 0], rich3[:, :, 1])
            nc.vector.tensor_add(tsum, tsum, rich3[:, :, 2])
            nc.vector.tensor_scalar(osb, tsum, 1.0 / C, None, ALU.mult)

            # ---- output DMAs ----
            for t in range(T_BLKS):
                b = t // (T_BLKS // B_CORE)
                hp0 = (t % (T_BLKS // B_CORE)) * 4
                nc.sync.dma_start(ov[b, hp0:hp0 + 4], osb[:, t:t + 1])

    nc.compile()
    return nc


@functools.lru_cache(maxsize=4)
def _build_cached(dct_flat: tuple) -> bass.Bass:
    return _build(dct_flat)


def kernel(x, dct_matrix):
    x = np.ascontiguousarray(np.asarray(x, dtype=np.float32))
    D = np.asarray(dct_matrix, dtype=np.float32)
    assert x.shape == (B_FULL, C, H, W), x.shape
    nc = _build_cached(tuple(float(v) for v in D.flatten()))
    import ml_dtypes
    ident_bf = np.eye(P, dtype=np.float32).astype(ml_dtypes.bfloat16)
    in_maps = [
        {"x": np.ascontiguousarray(x[i * B_CORE:(i + 1) * B_CORE]),
         "ident": ident_bf}
        for i in range(N_CORES)
    ]
    res = bass_utils.run_bass_kernel_spmd(
        nc, in_maps, core_ids=list(range(N_CORES)))
    out = np.concatenate([r["out"] for r in res.results], axis=0)
    return out.astype(np.float32)
